# revision 5
# baseline (speedup 1.0000x reference)
"""Multi-head attention (B=384, S=128, E=512, H=4, D=128) on 8 TRN2 NeuronCores.

Data-parallel: batch 384 -> 48 per core, projection weights replicated.

Layout/dtype decisions (vs the TRN2 matmul cost model: time = N_free x
cyc/row; fp32r is 1 cyc/row only at N>=256 and blocks fast-weight-load;
fp16/bf16 are 1 cyc/row always and get FWL):

  - The host feeds x ALREADY TRANSPOSED per core (xT[chunk, e, (j, s)],
    fp16, 2KB DMA lines): zero PE transposes and half the input DMA bytes.
  - All four projection weights are fed as fp16; every projection matmul
    runs fp16 (1 cyc/row, LDWEIGHTS ~97ns fully hidden under 213ns MMs).
    fp32 accumulation in PSUM throughout.
  - Scores are computed TRANSPOSED: ST[t,(h,s)] = matmul(lhsT=kT, rhs=qT),
    so exp(ST) on ScalarE writes the post-softmax weights wT straight to
    SBUF in the layout the AV matmul needs as rhs -- no PE w-transpose, no
    PSUM->SBUF copy for w at all.
  - Softmax normalization is deferred past the AV matmul: denom[s] =
    ones^T @ exp(ST) as a matmul whose M=128 replicates the row sums onto
    every partition (same N=512 cost as M=1), then VectorE does
    reciprocal + one fused multiply during the attT PSUM->SBUF copy.
    No max-subtraction: |S| < 88 so bf16 exp cannot overflow, and the
    unnormalized attT (< ~1e30) stays inside fp32.
  - exp weights in bf16 (need fp32 exponent range), v in bf16, attT in
    fp16 -> O projection fp16.

Per-iteration emission (engine streams execute in emission order):
  scoresT+exp(k) | xT-DMA(k+2) | QK-proj(k+1) | denom(k) | V-proj(k+1)
  | AV(k) | O-proj(k)
which keeps the PE stream dense while exp/recip/bias-adds drain on
ACT/DVE behind the next stage's matmuls. Dummy bf16 matmuls warm the PE
HAM clock-gate during the initial weight/x DMA window.
"""

import numpy as np

import concourse.bass as bass
import concourse.tile as tile
import concourse.mybir as mybir
from concourse import bacc
from concourse.bass_utils import run_bass_kernel_spmd

B, S, E, H, D = 384, 128, 512, 4, 128
NCORES = 8
BLOC = B // NCORES  # 48 batches per core
NB = 4  # batches per chunk
NCHUNK = BLOC // NB
NBS = NB * S  # 512 rows of x per chunk
EC = E // 128  # 4 chunks of the embed dim

F32 = mybir.dt.float32
BF16 = mybir.dt.bfloat16
F16 = mybir.dt.float16

_CACHE = {}


def build():
    nc = bacc.Bacc("TRN2", target_bir_lowering=False, debug=False, num_devices=NCORES)

    # x arrives pre-transposed fp16: xT[chunk, e, j*S + s] = x[chunk*NB+j, s, e]
    x = nc.dram_tensor("x", [NCHUNK, E, NBS], F16, kind="ExternalInput").ap()
    wq = nc.dram_tensor("Wq", [E, E], F16, kind="ExternalInput").ap()
    wk = nc.dram_tensor("Wk", [E, E], F16, kind="ExternalInput").ap()
    wv = nc.dram_tensor("Wv", [E, E], F16, kind="ExternalInput").ap()
    wo = nc.dram_tensor("Wo", [E, E], F16, kind="ExternalInput").ap()
    bq = nc.dram_tensor("bq", [E], F32, kind="ExternalInput").ap()
    bk = nc.dram_tensor("bk", [E], F32, kind="ExternalInput").ap()
    bv = nc.dram_tensor("bv", [E], F32, kind="ExternalInput").ap()
    bo = nc.dram_tensor("bo", [E], F32, kind="ExternalInput").ap()
    out = nc.dram_tensor("out", [BLOC, S, E], F32, kind="ExternalOutput").ap()

    with tile.TileContext(nc) as tc:
        with (
            tc.tile_pool(name="singles", bufs=1) as singles,
            tc.tile_pool(name="xp", bufs=2) as xp,
            tc.tile_pool(name="qkv", bufs=2) as qkv,
            tc.tile_pool(name="attn", bufs=2) as attn,
            tc.tile_pool(name="wsm", bufs=3) as wsm,
            tc.tile_pool(name="ps", bufs=8, space="PSUM") as ps,
        ):
            # Warm the PE HAM clock-gate immediately (PE would otherwise
            # idle through the initial DMA window and start cold at half
            # clock). Emitted first so it only depends on one DVE memset.
            dummy_bf = singles.tile([128, E], BF16, tag="dummy")
            nc.vector.memset(dummy_bf, 0.0)
            ones_bf = singles.tile([128, 128], BF16, tag="ones")
            nc.vector.memset(ones_bf, 1.0)
            warm_ps = ps.tile([128, E], F32, tag="ps", name="warm")
            for _ in range(18):
                nc.tensor.matmul(warm_ps, ones_bf[:], dummy_bf, start=True, stop=True)

            w_sb = {}
            w_dram = {"q": wq, "k": wk, "v": wv, "o": wo}
            for name in ("q", "k", "v", "o"):
                w_sb[name] = singles.tile([128, EC, E], F16, tag=f"w{name}", name=f"w{name}")

            def load_weight(name):
                for c in range(EC):
                    nc.sync.dma_start(
                        out=w_sb[name][:, c, :],
                        in_=w_dram[name][c * 128 : (c + 1) * 128, :],
                    )

            bq_sb = singles.tile([128, EC], F32, tag="bq")
            bk_sb = singles.tile([128, EC], F32, tag="bk")
            bv_sb = singles.tile([128, E], F32, tag="bv")
            bo_sb = singles.tile([128, E], F32, tag="bo")

            def load_biases():
                for t, b in ((bq_sb, bq), (bk_sb, bk)):
                    nc.sync.dma_start(
                        out=t,
                        in_=bass.AP(tensor=b.tensor, offset=0, ap=[[1, 128], [128, EC]]),
                    )
                for t, b in ((bv_sb, bv), (bo_sb, bo)):
                    nc.sync.dma_start(
                        out=t,
                        in_=bass.AP(tensor=b.tensor, offset=0, ap=[[0, 128], [1, E]]),
                    )

            def load_xt(chunk):
                """DMA one chunk's pre-transposed fp16 x: EC tiles of [128, NBS]."""
                xt = []
                for c in range(EC):
                    t = xp.tile([128, NBS], F16, tag=f"xt{c}")
                    nc.sync.dma_start(out=t, in_=x[chunk, c * 128 : (c + 1) * 128, :])
                    xt.append(t)
                return xt

            def proj_qk(xt):
                """QT/KT projections from xT, bias-added into fp16 tiles."""
                qt, kt = [], []
                for h in range(H):
                    p = ps.tile([128, NBS], F32, tag="ps")
                    for c in range(EC):
                        nc.tensor.matmul(
                            p,
                            w_sb["q"][:, c, h * 128 : (h + 1) * 128],
                            xt[c],
                            start=(c == 0),
                            stop=(c == EC - 1),
                        )
                    t = qkv.tile([128, NBS], F16, tag=f"qt{h}")
                    nc.scalar.add(out=t, in_=p, add=bq_sb[:, h : h + 1])
                    qt.append(t)
                    p = ps.tile([128, NBS], F32, tag="ps")
                    for c in range(EC):
                        nc.tensor.matmul(
                            p,
                            w_sb["k"][:, c, h * 128 : (h + 1) * 128],
                            xt[c],
                            start=(c == 0),
                            stop=(c == EC - 1),
                        )
                    t = qkv.tile([128, NBS], F16, tag=f"kt{h}")
                    nc.scalar.add(out=t, in_=p, add=bk_sb[:, h : h + 1])
                    kt.append(t)
                return qt, kt

            def proj_v(xt):
                """V projection (natural layout), bias-added into bf16 tiles."""
                v_sb = []
                for j in range(NB):
                    p = ps.tile([128, E], F32, tag="ps")
                    for c in range(EC):
                        nc.tensor.matmul(
                            p,
                            xt[c][:, j * 128 : (j + 1) * 128],
                            w_sb["v"][:, c, :],
                            start=(c == 0),
                            stop=(c == EC - 1),
                        )
                    t = qkv.tile([128, E], BF16, tag=f"v{j}")
                    nc.vector.tensor_add(out=t, in0=p, in1=bv_sb)
                    v_sb.append(t)
                return v_sb

            def scores_exp(qt, kt):
                """Transposed scores ST[t,(h,s)] then exp -> bf16 wT in SBUF."""
                wts = []
                for j in range(NB):
                    ps_s = ps.tile([128, H, 128], F32, tag="ps")
                    for h in range(H):
                        nc.tensor.matmul(
                            ps_s[:, h, :],
                            kt[h][:, j * 128 : (j + 1) * 128],
                            qt[h][:, j * 128 : (j + 1) * 128],
                            start=True,
                            stop=True,
                        )
                    wt = wsm.tile([128, H, 128], BF16, tag=f"wt{j}")
                    nc.scalar.activation(
                        out=wt,
                        in_=ps_s,
                        func=mybir.ActivationFunctionType.Exp,
                        bias=0.0,
                        scale=1.0,
                    )
                    wts.append(wt)
                return wts

            def denom(wts):
                """Row sums of exp, replicated to all partitions via ones-matmul,
                then reciprocal -> [128,(h,s)] f32 scale tiles."""
                rbs = []
                for j in range(NB):
                    dp = ps.tile([128, H, 128], F32, tag="ps")
                    nc.tensor.matmul(dp, ones_bf[:], wts[j][:, :, :], start=True, stop=True)
                    rb = wsm.tile([128, H, 128], F32, tag=f"rb{j}")
                    nc.vector.reciprocal(out=rb, in_=dp)
                    rbs.append(rb)
                return rbs

            def av(wts, rbs, v_sb):
                """attT = v^T-form @ wT, normalized during the PSUM->SBUF copy."""
                ats = []
                for j in range(NB):
                    ps_at = ps.tile([128, H, 128], F32, tag="ps")
                    for h in range(H):
                        nc.tensor.matmul(
                            ps_at[:, h, :],
                            v_sb[j][:, h * 128 : (h + 1) * 128],
                            wts[j][:, h, :],
                            start=True,
                            stop=True,
                        )
                    at = attn.tile([128, H, 128], F16, tag=f"at{j}")
                    nc.vector.tensor_mul(out=at, in0=ps_at, in1=rbs[j])
                    ats.append(at)
                return ats

            def oproj(chunk, ats):
                b0 = chunk * NB
                for j in range(NB):
                    p = ps.tile([128, E], F32, tag="ps")
                    for h in range(H):
                        nc.tensor.matmul(
                            p,
                            ats[j][:, h, :],
                            w_sb["o"][:, h, :],
                            start=(h == 0),
                            stop=(h == H - 1),
                        )
                    o_sb = attn.tile([128, E], F32, tag=f"o{j}")
                    nc.vector.tensor_add(out=o_sb, in0=p, in1=bo_sb)
                    nc.sync.dma_start(out=out[b0 + j], in_=o_sb)

            # Startup: xt(0) + Wq/Wk first so the first projections can
            # begin ASAP; remaining weights/biases behind them.
            xts = {0: load_xt(0)}
            load_weight("q")
            load_weight("k")
            load_biases()
            load_weight("v")
            load_weight("o")
            states = {0: proj_qk(xts[0])}
            vs = {0: proj_v(xts[0])}
            xts[1] = load_xt(1) if NCHUNK > 1 else None
            for k in range(NCHUNK):
                wts = scores_exp(*states[k])
                if k + 2 < NCHUNK:
                    xts[k + 2] = load_xt(k + 2)
                if k + 1 < NCHUNK:
                    states[k + 1] = proj_qk(xts[k + 1])
                rbs = denom(wts)
                if k + 1 < NCHUNK:
                    vs[k + 1] = proj_v(xts[k + 1])
                ats = av(wts, rbs, vs[k])
                oproj(k, ats)

    nc.compile()
    return nc


def make_in_maps(inputs):
    x = np.ascontiguousarray(np.asarray(inputs["x"], dtype=np.float32))
    # Pre-transpose per core: [BLOC, S, E] -> [NCHUNK, E, NB*S], fp16.
    xt_all = np.ascontiguousarray(
        x.reshape(NCORES, NCHUNK, NB, S, E)
        .transpose(0, 1, 4, 2, 3)
        .reshape(NCORES, NCHUNK, E, NB * S)
        .astype(np.float16)
    )
    shared = {
        k: np.ascontiguousarray(np.asarray(inputs[k]).astype(np.float16))
        for k in ("Wq", "Wk", "Wv", "Wo")
    }
    shared.update(
        {
            k: np.ascontiguousarray(np.asarray(inputs[k], dtype=np.float32))
            for k in ("bq", "bk", "bv", "bo")
        }
    )
    return [{"x": xt_all[i], **shared} for i in range(NCORES)]


def kernel(**inputs):
    if "nc" not in _CACHE:
        _CACHE["nc"] = build()
    nc = _CACHE["nc"]
    in_maps = make_in_maps(inputs)
    res = run_bass_kernel_spmd(nc, in_maps, core_ids=list(range(NCORES)))
    return np.concatenate([res.results[i]["out"] for i in range(NCORES)], axis=0)


# revision 6
# speedup vs baseline: 1.9898x; 1.9898x over previous
"""Multi-head attention (B=384, S=128, E=512, H=4, D=128) on 8 TRN2 NeuronCores.

Data-parallel: batch 384 -> 48 per core, projection weights replicated.

Layout/dtype decisions (vs the TRN2 matmul cost model: time = N_free x
cyc/row; fp32r is 1 cyc/row only at N>=256 and blocks fast-weight-load;
fp16/bf16 are 1 cyc/row always and get FWL):

  - The host feeds x ALREADY TRANSPOSED per core (xT[chunk, e, (j, s)],
    fp16, 2KB DMA lines): zero PE transposes and half the input DMA bytes.
  - All four projection weights are fed as fp16; every projection matmul
    runs fp16 (1 cyc/row, LDWEIGHTS ~97ns fully hidden under 213ns MMs).
    fp32 accumulation in PSUM throughout.
  - Scores are computed TRANSPOSED: ST[t,(h,s)] = matmul(lhsT=kT, rhs=qT),
    so exp(ST) on ScalarE writes the post-softmax weights wT straight to
    SBUF in the layout the AV matmul needs as rhs -- no PE w-transpose, no
    PSUM->SBUF copy for w at all.
  - Softmax normalization is deferred past the AV matmul: denom[s] =
    ones^T @ exp(ST) as a matmul whose M=128 replicates the row sums onto
    every partition (same N=512 cost as M=1), then VectorE does
    reciprocal + one fused multiply during the attT PSUM->SBUF copy.
    No max-subtraction: |S| < 88 so bf16 exp cannot overflow, and the
    unnormalized attT (< ~1e30) stays inside fp32.
  - exp weights in bf16 (need fp32 exponent range), v in bf16, attT in
    fp16 -> O projection fp16.

Per-iteration emission (engine streams execute in emission order):
  scoresT+exp(k) | xT-DMA(k+2) | QK-proj(k+1) | denom(k) | V-proj(k+1)
  | AV(k) | O-proj(k)
which keeps the PE stream dense while exp/recip/bias-adds drain on
ACT/DVE behind the next stage's matmuls. Dummy bf16 matmuls warm the PE
HAM clock-gate during the initial weight/x DMA window.
"""

import numpy as np

import concourse.bass as bass
import concourse.tile as tile
import concourse.mybir as mybir
from concourse import bacc
from concourse.bass_utils import run_bass_kernel_spmd

B, S, E, H, D = 384, 128, 512, 4, 128
NCORES = 8
BLOC = B // NCORES  # 48 batches per core
NB = 4  # batches per chunk
NCHUNK = BLOC // NB
NBS = NB * S  # 512 rows of x per chunk
EC = E // 128  # 4 chunks of the embed dim

F32 = mybir.dt.float32
BF16 = mybir.dt.bfloat16
F16 = mybir.dt.float16

_CACHE = {}


def build():
    nc = bacc.Bacc("TRN2", target_bir_lowering=False, debug=False, num_devices=NCORES)

    # x arrives pre-transposed fp16: xT[chunk, e, j*S + s] = x[chunk*NB+j, s, e]
    x = nc.dram_tensor("x", [NCHUNK, E, NBS], F16, kind="ExternalInput").ap()
    wq = nc.dram_tensor("Wq", [E, E], F16, kind="ExternalInput").ap()
    wk = nc.dram_tensor("Wk", [E, E], F16, kind="ExternalInput").ap()
    wv = nc.dram_tensor("Wv", [E, E], F16, kind="ExternalInput").ap()
    wo = nc.dram_tensor("Wo", [E, E], F16, kind="ExternalInput").ap()
    bq = nc.dram_tensor("bq", [E], F32, kind="ExternalInput").ap()
    bk = nc.dram_tensor("bk", [E], F32, kind="ExternalInput").ap()
    bv = nc.dram_tensor("bv", [E], F32, kind="ExternalInput").ap()
    bo = nc.dram_tensor("bo", [E], F32, kind="ExternalInput").ap()
    out = nc.dram_tensor("out", [BLOC, S, E], F32, kind="ExternalOutput").ap()

    with tile.TileContext(nc) as tc:
        with (
            tc.tile_pool(name="singles", bufs=1) as singles,
            tc.tile_pool(name="xp", bufs=2) as xp,
            tc.tile_pool(name="qkv", bufs=2) as qkv,
            tc.tile_pool(name="attn", bufs=2) as attn,
            tc.tile_pool(name="wsm", bufs=3) as wsm,
            tc.tile_pool(name="ps", bufs=8, space="PSUM") as ps,
        ):
            # Warm the PE HAM clock-gate immediately (PE would otherwise
            # idle through the initial DMA window and start cold at half
            # clock). Emitted first so it only depends on one DVE memset.
            dummy_bf = singles.tile([128, E], BF16, tag="dummy")
            nc.vector.memset(dummy_bf, 0.0)
            ones_bf = singles.tile([128, 128], BF16, tag="ones")
            nc.vector.memset(ones_bf, 1.0)
            warm_ps = ps.tile([128, E], F32, tag="ps", name="warm")
            for _ in range(18):
                nc.tensor.matmul(warm_ps, ones_bf[:], dummy_bf, start=True, stop=True)

            w_sb = {}
            w_dram = {"q": wq, "k": wk, "v": wv, "o": wo}
            for name in ("q", "k", "v", "o"):
                w_sb[name] = singles.tile([128, EC, E], F16, tag=f"w{name}", name=f"w{name}")

            def load_weight(name):
                for c in range(EC):
                    nc.sync.dma_start(
                        out=w_sb[name][:, c, :],
                        in_=w_dram[name][c * 128 : (c + 1) * 128, :],
                    )

            bq_sb = singles.tile([128, EC], F32, tag="bq")
            bk_sb = singles.tile([128, EC], F32, tag="bk")
            bv_sb = singles.tile([128, E], F32, tag="bv")
            bo_sb = singles.tile([128, E], F32, tag="bo")

            def load_biases():
                for t, b in ((bq_sb, bq), (bk_sb, bk)):
                    nc.sync.dma_start(
                        out=t,
                        in_=bass.AP(tensor=b.tensor, offset=0, ap=[[1, 128], [128, EC]]),
                    )
                for t, b in ((bv_sb, bv), (bo_sb, bo)):
                    nc.sync.dma_start(
                        out=t,
                        in_=bass.AP(tensor=b.tensor, offset=0, ap=[[0, 128], [1, E]]),
                    )

            def load_xt(chunk):
                """DMA one chunk's pre-transposed fp16 x: EC tiles of [128, NBS]."""
                xt = []
                for c in range(EC):
                    t = xp.tile([128, NBS], F16, tag=f"xt{c}")
                    nc.sync.dma_start(out=t, in_=x[chunk, c * 128 : (c + 1) * 128, :])
                    xt.append(t)
                return xt

            def proj_qk(xt):
                """QT/KT projections from xT, bias-added into fp16 tiles."""
                qt, kt = [], []
                for h in range(H):
                    p = ps.tile([128, NBS], F32, tag="ps")
                    for c in range(EC):
                        nc.tensor.matmul(
                            p,
                            w_sb["q"][:, c, h * 128 : (h + 1) * 128],
                            xt[c],
                            start=(c == 0),
                            stop=(c == EC - 1),
                        )
                    t = qkv.tile([128, NBS], F16, tag=f"qt{h}")
                    nc.scalar.add(out=t, in_=p, add=bq_sb[:, h : h + 1])
                    qt.append(t)
                    p = ps.tile([128, NBS], F32, tag="ps")
                    for c in range(EC):
                        nc.tensor.matmul(
                            p,
                            w_sb["k"][:, c, h * 128 : (h + 1) * 128],
                            xt[c],
                            start=(c == 0),
                            stop=(c == EC - 1),
                        )
                    t = qkv.tile([128, NBS], F16, tag=f"kt{h}")
                    nc.scalar.add(out=t, in_=p, add=bk_sb[:, h : h + 1])
                    kt.append(t)
                return qt, kt

            def proj_v(xt):
                """V projection (natural layout), bias-added into bf16 tiles."""
                v_sb = []
                for j in range(NB):
                    p = ps.tile([128, E], F32, tag="ps")
                    for c in range(EC):
                        nc.tensor.matmul(
                            p,
                            xt[c][:, j * 128 : (j + 1) * 128],
                            w_sb["v"][:, c, :],
                            start=(c == 0),
                            stop=(c == EC - 1),
                        )
                    t = qkv.tile([128, E], BF16, tag=f"v{j}")
                    nc.vector.tensor_add(out=t, in0=p, in1=bv_sb)
                    v_sb.append(t)
                return v_sb

            def scores_exp(qt, kt):
                """Transposed scores ST[t,(h,s)] then exp -> bf16 wT in SBUF."""
                wts = []
                for j in range(NB):
                    ps_s = ps.tile([128, H, 128], F32, tag="ps")
                    for h in range(H):
                        nc.tensor.matmul(
                            ps_s[:, h, :],
                            kt[h][:, j * 128 : (j + 1) * 128],
                            qt[h][:, j * 128 : (j + 1) * 128],
                            start=True,
                            stop=True,
                        )
                    wt = wsm.tile([128, H, 128], BF16, tag=f"wt{j}")
                    nc.scalar.activation(
                        out=wt,
                        in_=ps_s,
                        func=mybir.ActivationFunctionType.Exp,
                        bias=0.0,
                        scale=1.0,
                    )
                    wts.append(wt)
                return wts

            def denom(wts):
                """Row sums of exp, replicated to all partitions via ones-matmul,
                then reciprocal -> [128,(h,s)] f32 scale tiles."""
                rbs = []
                for j in range(NB):
                    dp = ps.tile([128, H, 128], F32, tag="ps")
                    nc.tensor.matmul(dp, ones_bf[:], wts[j][:, :, :], start=True, stop=True)
                    rb = wsm.tile([128, H, 128], F32, tag=f"rb{j}")
                    # ~18-bit approx reciprocal: one DVE op (~0.8us) vs ~4us
                    # for exact reciprocal at this size; denominators are in
                    # [1, ~1e32] so the seed's edge cases can't occur.
                    nc.vector.reciprocal_approx_fast(out=rb, in_=dp)
                    rbs.append(rb)
                return rbs

            def av(wts, rbs, v_sb):
                """attT = v^T-form @ wT, normalized during the PSUM->SBUF copy."""
                ats = []
                for j in range(NB):
                    ps_at = ps.tile([128, H, 128], F32, tag="ps")
                    for h in range(H):
                        nc.tensor.matmul(
                            ps_at[:, h, :],
                            v_sb[j][:, h * 128 : (h + 1) * 128],
                            wts[j][:, h, :],
                            start=True,
                            stop=True,
                        )
                    at = attn.tile([128, H, 128], F16, tag=f"at{j}")
                    nc.vector.tensor_mul(out=at, in0=ps_at, in1=rbs[j])
                    ats.append(at)
                return ats

            def oproj(chunk, ats):
                b0 = chunk * NB
                for j in range(NB):
                    p = ps.tile([128, E], F32, tag="ps")
                    for h in range(H):
                        nc.tensor.matmul(
                            p,
                            ats[j][:, h, :],
                            w_sb["o"][:, h, :],
                            start=(h == 0),
                            stop=(h == H - 1),
                        )
                    o_sb = attn.tile([128, E], F32, tag=f"o{j}")
                    nc.vector.tensor_add(out=o_sb, in0=p, in1=bo_sb)
                    nc.sync.dma_start(out=out[b0 + j], in_=o_sb)

            # Startup: xt(0) + Wq/Wk first so the first projections can
            # begin ASAP; remaining weights/biases behind them.
            xts = {0: load_xt(0)}
            load_weight("q")
            load_weight("k")
            load_biases()
            load_weight("v")
            load_weight("o")
            states = {0: proj_qk(xts[0])}
            vs = {0: proj_v(xts[0])}
            xts[1] = load_xt(1) if NCHUNK > 1 else None
            for k in range(NCHUNK):
                wts = scores_exp(*states[k])
                if k + 2 < NCHUNK:
                    xts[k + 2] = load_xt(k + 2)
                if k + 1 < NCHUNK:
                    states[k + 1] = proj_qk(xts[k + 1])
                rbs = denom(wts)
                if k + 1 < NCHUNK:
                    vs[k + 1] = proj_v(xts[k + 1])
                ats = av(wts, rbs, vs[k])
                oproj(k, ats)

    nc.compile()
    return nc


def make_in_maps(inputs):
    x = np.ascontiguousarray(np.asarray(inputs["x"], dtype=np.float32))
    # Pre-transpose per core: [BLOC, S, E] -> [NCHUNK, E, NB*S], fp16.
    xt_all = np.ascontiguousarray(
        x.reshape(NCORES, NCHUNK, NB, S, E)
        .transpose(0, 1, 4, 2, 3)
        .reshape(NCORES, NCHUNK, E, NB * S)
        .astype(np.float16)
    )
    shared = {
        k: np.ascontiguousarray(np.asarray(inputs[k]).astype(np.float16))
        for k in ("Wq", "Wk", "Wv", "Wo")
    }
    shared.update(
        {
            k: np.ascontiguousarray(np.asarray(inputs[k], dtype=np.float32))
            for k in ("bq", "bk", "bv", "bo")
        }
    )
    return [{"x": xt_all[i], **shared} for i in range(NCORES)]


def kernel(**inputs):
    if "nc" not in _CACHE:
        _CACHE["nc"] = build()
    nc = _CACHE["nc"]
    in_maps = make_in_maps(inputs)
    res = run_bass_kernel_spmd(nc, in_maps, core_ids=list(range(NCORES)))
    return np.concatenate([res.results[i]["out"] for i in range(NCORES)], axis=0)
